# revision 9
# baseline (speedup 1.0000x reference)
"""GCN v6: dense fp8 aggregation matmul - no dst padding, 10KB descriptors.

Same architecture as v4.3 (dense per-core fp8 count matrix S8, DoubleRow
chunk-pair matmuls accumulating aggT in PSUM, exact fp32 recip at
PSUM->SBUF, bf16 epilogue GEMMs, fp16 output), with two stream trims:

- dst axis uses 2500 slots/core (20000/8 exactly - no padded columns;
  the old 2560 wasted 2.3% of the dominant S stream). Epilogue windows
  are 500 wide but PLACED at 512-col PSUM offsets so each matmul output
  stays inside one 2KB PSUM bank.
- S tiles carry 2 chunk-pairs (10000B/partition descriptors vs 5120B),
  cutting per-descriptor overhead ~4%.

DoubleRow subtile stride inside a tile is 2500B - safely off the 2048B
SBUF bank-conflict stride. ~53MB/core HBM traffic.
"""

import numpy as np

N_NODES = 20000
D = 128
N_CORES = 8
NPC = N_NODES // N_CORES          # 2500 dst slots per core (exact)
W = 500                           # epilogue window width
TPT = NPC // W                    # 5 windows
BANK = 512                        # fp32 words per PSUM bank (window stride)
N_SRC = 20480                     # padded src axis (80 chunk-pairs)
NCP = N_SRC // 256                # 80
BDT = 2                           # chunk-pairs per S DMA tile
NT = NCP // BDT                   # 40 tiles

_prog_cache = {}


def _build_program6():
    import concourse.mybir as mybir
    from concourse import bacc
    from concourse.tile import TileContext

    dt = mybir.dt
    DR = mybir.MatmulPerfMode.DoubleRow
    nc = bacc.Bacc()

    h8 = nc.declare_dram_parameter("h8", [128, N_SRC], dt.float8e4, isOutput=False)
    smat = nc.declare_dram_parameter(
        "smat", [128, NCP * 2 * NPC], dt.float8e4, isOutput=False
    )
    hT = nc.declare_dram_parameter("hT", [D, NPC], dt.bfloat16, isOutput=False)
    recip = nc.declare_dram_parameter("recip", [1, NPC], dt.float32, isOutput=False)
    wselfT = nc.declare_dram_parameter("wselfT", [D, D], dt.bfloat16, isOutput=False)
    wneiT = nc.declare_dram_parameter("wneiT", [D, D], dt.bfloat16, isOutput=False)
    bself = nc.declare_dram_parameter("bself", [D, 1], dt.float32, isOutput=False)
    outT = nc.declare_dram_parameter("outT", [D, NPC], dt.float16, isOutput=True)

    with (
        TileContext(nc) as tc,
        tc.tile_pool(name="const", bufs=1) as cpool,
        tc.tile_pool(name="sel", bufs=5) as spool,
        tc.tile_pool(name="agg", bufs=3) as apool,
        tc.tile_pool(name="res", bufs=3) as opool,
        tc.tile_pool(name="pagg", bufs=1, space="PSUM") as pagg,
        tc.tile_pool(name="pout", bufs=2, space="PSUM") as pout,
    ):
        h8_sb = cpool.tile([128, N_SRC], dt.float8e4)
        nc.sync.dma_start(out=h8_sb[:], in_=h8[:])
        hT_sb = cpool.tile([D, NPC], dt.bfloat16)
        nc.sync.dma_start(out=hT_sb[:], in_=hT[:])
        recip_sb = cpool.tile([128, NPC], dt.float32)
        nc.sync.dma_start(out=recip_sb[:], in_=recip[:, :].to_broadcast([128, NPC]))
        wselfT_sb = cpool.tile([D, D], dt.bfloat16)
        nc.sync.dma_start(out=wselfT_sb[:], in_=wselfT[:])
        wneiT_sb = cpool.tile([D, D], dt.bfloat16)
        nc.sync.dma_start(out=wneiT_sb[:], in_=wneiT[:])
        bself_sb = cpool.tile([D, 1], dt.float32)
        nc.sync.dma_start(out=bself_sb[:], in_=bself[:])

        # [128, cp, 2, 128]: row (cp*256 + i*128 + p) of padded h, fp8
        h8r = h8_sb.rearrange("p (cp two m) -> p cp two m", two=2, m=128)

        # pa: 5 windows of 500 fp32, each anchored at a 512-word bank offset
        pa = pagg.tile([128, TPT * BANK], dt.float32)
        for t in range(NT):
            s = spool.tile([128, BDT * 2 * NPC], dt.float8e4)
            nc.sync.dma_start(
                out=s[:], in_=smat[:, t * BDT * 2 * NPC : (t + 1) * BDT * 2 * NPC]
            )
            sr = s.rearrange("p (c two n) -> p c two n", c=BDT, two=2)
            for j in range(BDT):
                cp = t * BDT + j
                for k in range(TPT):
                    nc.tensor.matmul(
                        out=pa[:, k * BANK : k * BANK + W],
                        lhsT=h8r[:, cp, :, :],
                        rhs=sr[:, j, :, k * W : (k + 1) * W],
                        start=(cp == 0),
                        stop=(cp == NCP - 1),
                        perf_mode=DR,
                    )

        for k in range(TPT):
            psl = slice(k * BANK, k * BANK + W)
            csl = slice(k * W, (k + 1) * W)
            aggT = apool.tile([128, W], dt.bfloat16)
            nc.vector.tensor_mul(out=aggT[:], in0=pa[:, psl], in1=recip_sb[:, csl])
            po = pout.tile([128, W], dt.float32, space="PSUM")
            nc.tensor.matmul(
                out=po[:], lhsT=wselfT_sb[:], rhs=hT_sb[:, csl], start=True, stop=False
            )
            nc.tensor.matmul(
                out=po[:], lhsT=wneiT_sb[:], rhs=aggT[:], start=False, stop=True
            )
            o = opool.tile([128, W], dt.float16)
            nc.scalar.activation(
                out=o[:],
                in_=po[:],
                func=mybir.ActivationFunctionType.Relu,
                bias=bself_sb[:, :1],
            )
            nc.gpsimd.dma_start(out=outT[:, csl], in_=o[:])

    nc.compile()
    return nc


def _host_prep(h, edge_index, deg):
    import ml_dtypes

    f8 = ml_dtypes.float8_e4m3
    bf16 = ml_dtypes.bfloat16

    src = np.asarray(edge_index[0], dtype=np.int64)
    dst = np.asarray(edge_index[1], dtype=np.int64)
    h = np.asarray(h, dtype=np.float32)
    deg = np.asarray(deg, dtype=np.float32)

    h_pad = np.zeros((N_SRC, D), np.float32)
    h_pad[:N_NODES] = h
    h8_flat = (
        h_pad.astype(f8).reshape(NCP, 2, 128, D).transpose(2, 0, 1, 3).reshape(128, -1)
    )
    h8_flat = np.ascontiguousarray(h8_flat)

    recip = (1.0 / np.maximum(deg, 1.0)).astype(np.float32)

    lut = np.arange(256).astype(np.float32).astype(f8)

    core_of_dst = dst // NPC
    order = np.argsort(core_of_dst, kind="stable")
    src_s, dst_s = src[order], dst[order]
    bounds = np.searchsorted(core_of_dst[order], np.arange(N_CORES + 1))

    per_core = []
    for cc in range(N_CORES):
        lo, hi = bounds[cc], bounds[cc + 1]
        s_u8 = np.zeros((N_SRC, NPC), np.uint8)
        np.add.at(s_u8, (src_s[lo:hi], dst_s[lo:hi] - cc * NPC), 1)
        s8 = lut[s_u8]
        s8 = s8.reshape(NCP, 2, 128, NPC).transpose(2, 0, 1, 3).reshape(128, -1)
        per_core.append(np.ascontiguousarray(s8))

    hT_bf = np.ascontiguousarray(h.T.astype(bf16))
    return h8_flat, per_core, recip, hT_bf


def kernel(h, edge_index, deg, w_self, b_self, w_nei):
    import os

    import ml_dtypes
    from concourse.bass_utils import run_bass_kernel_spmd

    bf16 = ml_dtypes.bfloat16

    h8_flat, per_core, recip, hT_bf = _host_prep(h, edge_index, deg)

    wselfT = np.ascontiguousarray(np.asarray(w_self, dtype=np.float32).T.astype(bf16))
    wneiT = np.ascontiguousarray(np.asarray(w_nei, dtype=np.float32).T.astype(bf16))
    b_col = np.ascontiguousarray(np.asarray(b_self, dtype=np.float32).reshape(D, 1))

    in_maps = []
    for cc in range(N_CORES):
        in_maps.append(
            {
                "h8": h8_flat,
                "smat": per_core[cc],
                "hT": np.ascontiguousarray(hT_bf[:, cc * NPC : (cc + 1) * NPC]),
                "recip": np.ascontiguousarray(
                    recip[cc * NPC : (cc + 1) * NPC].reshape(1, NPC)
                ),
                "wselfT": wselfT,
                "wneiT": wneiT,
                "bself": b_col,
            }
        )

    if "v6" not in _prog_cache:
        _prog_cache["v6"] = _build_program6()
    nc = _prog_cache["v6"]

    trace = bool(int(os.environ.get("GCN_TRACE", "0")))
    res = run_bass_kernel_spmd(nc, in_maps, core_ids=list(range(N_CORES)), trace=trace)
    kernel.last_results = res

    outT = np.concatenate([r["outT"] for r in res.results], axis=1)
    return np.ascontiguousarray(outT.T.astype(np.float32))
